# revision 6
# baseline (speedup 1.0000x reference)
"""Trainium2 Bass kernel for nn_DQN_57904749085018 (gnn_message_passing).

Computation (reference semantics):
    g   = x[:, idx]                                  [B, S, L] gather
    h   = (g - mean) * rsqrt(var+eps) * gamma + beta [B, S, L] batchnorm (eval)
    h1  = tanh(einsum('bsl,sol->bso', h, W1) + b1)   [B, S, 3]
    h2  = tanh(einsum('bsk,sok->bso', h1, W2) + b2)  [B, S, 2]
    a, sb = h2[..., 0], h2[..., 1]
    out[b,i,j] = tanh(a[b,i]*W3[i,j,0] + sb[b,j]*W3[i,j,1] + b3[i,j])
    -> reshape [B, S*S]

Kernel strategy (pure data parallel over 8 cores, batch-sharded):
  * gather + batchnorm + Linear1 fold into ONE dense matmul x @ Weff.T
    (Weff host-precomputed from idx/gamma/beta/mean/var/W1 - it is tiny).
  * the pairwise head is expressed as two block-structured matmuls:
      z = a' @ M0' + sb @ M1   with  M0'[k, i*S+j] = d_{k,i} W3[i,j,0],
      M0'[S, i*S+j] = b3[i,j] (ones-row trick), M1[k, i*S+j] = d_{k,j} W3[i,j,1]
    then out = tanh(z) on the scalar engine.
  * all matmuls run in float32r (full-rate PE); accumulation is fp32 in PSUM.
  * per 512-batch super-tile: PE-transpose x to feature-major, 2 small MLP
    stages, then 2 matmuls + 1 tanh + 1 DMA per 1024-column output chunk.
"""

import sys

import numpy as np

if "/opt/trn_rl_repo" not in sys.path:
    sys.path.insert(0, "/opt/trn_rl_repo")

import concourse.bacc as bacc
import concourse.mybir as mybir
from concourse import bass_utils
from concourse.masks import make_identity
from concourse.tile import TileContext

S = 100
L = 13
FEAT = 4 * S + 7  # 407
B = 8192
EPS = 1e-5
N_CORES = 8
BL = B // N_CORES  # 1024 batch rows per core
ST = 512  # batch super-tile (matmul moving dim)
N_ST = BL // ST  # 2
SS = S * S  # 10000
CHUNK = 1024  # output column chunk (2 PSUM banks)
CHUNKS = [(c * CHUNK, min(CHUNK, SS - c * CHUNK)) for c in range((SS + CHUNK - 1) // CHUNK)]
FCH = [(0, 128), (128, 128), (256, 128), (384, FEAT - 384)]  # feature chunks
F32R = mybir.dt.float32r
F32 = mybir.dt.float32

_module_cache = None


def _build_indices():
    idx = [[2 * i, 2 * i + 1] for i in range(S)]
    start = 2 * S
    for k in range(S):
        u, v = k, (k + 1) % S
        idx[u].extend([start, start + 1])
        idx[v].extend([start, start + 1])
        start += 2
    g0 = 4 * S
    for i in range(S):
        idx[i].extend(range(g0, g0 + 7))
    return np.asarray(idx, dtype=np.int64)


def _host_weights(inputs):
    f64 = np.float64
    gamma = np.asarray(inputs["gamma"], f64)
    beta = np.asarray(inputs["beta"], f64)
    mean = np.asarray(inputs["mean"], f64)
    var = np.asarray(inputs["var"], f64)
    W1 = np.asarray(inputs["W1"], f64)  # [S, 3, L]
    b1 = np.asarray(inputs["b1"], f64)  # [S, 3]
    W2 = np.asarray(inputs["W2"], f64)  # [S, 2, 3]
    b2 = np.asarray(inputs["b2"], f64)  # [S, 2]
    W3 = np.asarray(inputs["W3"], f64)  # [S, S, 2]
    b3 = np.asarray(inputs["b3"], f64)  # [S, S]
    idx = np.asarray(inputs["idx"], np.int64)  # [S, L]

    scale = gamma / np.sqrt(var + EPS)  # [S, L]
    shift = beta - mean * scale  # [S, L]

    # Weff[(s,o), f] = sum_l [idx[s,l]==f] W1[s,o,l]*scale[s,l]
    Wsc = W1 * scale[:, None, :]  # [S, 3, L]
    Weff = np.zeros((S, 3, FEAT), f64)
    s_ix = np.repeat(np.arange(S), 3 * L)
    o_ix = np.tile(np.repeat(np.arange(3), L), S)
    f_ix = np.repeat(idx[:, None, :], 3, axis=1).ravel()
    np.add.at(Weff, (s_ix, o_ix, f_ix), Wsc.ravel())
    Weff = Weff.reshape(3 * S, FEAT)
    beff = (b1 + np.einsum("sol,sl->so", W1, shift)).reshape(3 * S)

    # W2eff[(o2*S+s), (s*3+k)] = W2[s, o2, k]
    W2eff = np.zeros((2 * S, 3 * S), f64)
    for s in range(S):
        for o2 in range(2):
            W2eff[o2 * S + s, s * 3 : s * 3 + 3] = W2[s, o2, :]
    b2eff = b2.T.reshape(2 * S)  # [o2*S+s]

    # M0p[k, i*S+j] = d_{k,i} W3[i,j,0]; row S = b3 flat. M1[k, i*S+j] = d_{k,j} W3[i,j,1]
    M0p = np.zeros((S + 2, SS), f64)
    M1 = np.zeros((S, SS), f64)
    cols = np.arange(SS)
    M0p[np.repeat(np.arange(S), S), cols] = W3[:, :, 0].ravel()

    M1[np.tile(np.arange(S), S), cols] = W3[:, :, 1].ravel()

    def split_hl(v):
        # hi keeps 10 mantissa bits (exactly representable at fp32r ingestion);
        # lo carries the residual so the bias survives reduced-precision matmul.
        hi = np.asarray(v, np.float32).view(np.uint32) & np.uint32(0xFFFFE000)
        hi = hi.view(np.float32).astype(f64)
        return hi, np.asarray(v - hi)

    b3_hi, b3_lo = split_hl(b3.ravel())
    beff_hi, beff_lo = split_hl(beff)
    b2eff_hi, b2eff_lo = split_hl(b2eff)

    M0p[S, :] = b3_hi
    M0p[S + 1, :] = b3_lo
    c32 = lambda a: np.ascontiguousarray(a, dtype=np.float32)
    return {
        "wefft": c32(Weff.T),  # [FEAT, 300]
        "befft": c32(np.stack([beff_hi, beff_lo])),  # [2, 300]
        "w2efft": c32(W2eff.T),  # [300, 200]
        "b2efft": c32(np.stack([b2eff_hi, b2eff_lo])),  # [2, 200]
        "m0p": c32(M0p),  # [102, SS]
        "m1": c32(M1),  # [100, SS]
        "onesd": np.ones((2, 2 * ST), np.float32),
    }


def _build_module():
    global _module_cache
    if _module_cache is not None:
        return _module_cache

    nc = bacc.Bacc("TRN2", target_bir_lowering=False, debug=False, num_devices=N_CORES)
    xin = nc.dram_tensor("xin", [BL, FEAT], F32, kind="ExternalInput").ap()
    wefft = nc.dram_tensor("wefft", [FEAT, 300], F32R, kind="ExternalInput").ap()
    befft = nc.dram_tensor("befft", [2, 300], F32R, kind="ExternalInput").ap()
    w2efft = nc.dram_tensor("w2efft", [300, 200], F32R, kind="ExternalInput").ap()
    b2efft = nc.dram_tensor("b2efft", [2, 200], F32R, kind="ExternalInput").ap()
    m0p = nc.dram_tensor("m0p", [S + 2, SS], F32R, kind="ExternalInput").ap()
    m1 = nc.dram_tensor("m1", [S, SS], F32R, kind="ExternalInput").ap()
    onesd = nc.dram_tensor("onesd", [2, 2 * ST], F32R, kind="ExternalInput").ap()
    yout = nc.dram_tensor("yout", [BL, SS], F32, kind="ExternalOutput").ap()

    TANH = mybir.ActivationFunctionType.Tanh

    with TileContext(nc) as tc:
        with (
            tc.tile_pool(name="const", bufs=1) as const,
            tc.tile_pool(name="xin_pool", bufs=2) as xin_pool,
            tc.tile_pool(name="xt_pool", bufs=2) as xt_pool,
            tc.tile_pool(name="h1_pool", bufs=2) as h1_pool,
            tc.tile_pool(name="asb_pool", bufs=2) as asb_pool,
            tc.tile_pool(name="out_pool", bufs=4) as out_pool,
            tc.tile_pool(name="pt_pool", bufs=2, space="PSUM") as pt_pool,
            tc.tile_pool(name="pm1_pool", bufs=2, space="PSUM") as pm1_pool,
            tc.tile_pool(name="pf_pool", bufs=2, space="PSUM") as pf_pool,
        ):
            identity = const.tile([128, 128], F32)
            make_identity(nc, identity)
            ones2 = const.tile([2, 2 * ST], F32R)
            nc.sync.dma_start(ones2[:], onesd[:, :])
            ones = ones2[:, 0:ST]  # [2, ST] - K=2 rhs for hi/lo bias matmuls

            wefft_k = []
            for k, (f0, fw) in enumerate(FCH):
                t = const.tile([fw, 3 * S], F32R, name=f"wefft_{k}")
                nc.sync.dma_start(t[:], wefft[f0 : f0 + fw, :])
                wefft_k.append(t)
            befft_t = const.tile([2, 3 * S], F32R)
            nc.sync.dma_start(befft_t[:], befft[:, :])
            w2efft_k = []
            for k in range(3):
                t = const.tile([100, 2 * S], F32R, name=f"w2efft_{k}")
                nc.sync.dma_start(t[:], w2efft[k * 100 : (k + 1) * 100, :])
                w2efft_k.append(t)
            b2efft_t = const.tile([2, 2 * S], F32R)
            nc.sync.dma_start(b2efft_t[:], b2efft[:, :])
            m0c, m1c = [], []
            for c, (c0, cw) in enumerate(CHUNKS):
                t0 = const.tile([S + 2, cw], F32R, name=f"m0c_{c}")
                nc.sync.dma_start(t0[:], m0p[:, c0 : c0 + cw])
                m0c.append(t0)
                t1 = const.tile([S, cw], F32R, name=f"m1c_{c}")
                nc.sync.dma_start(t1[:], m1[:, c0 : c0 + cw])
                m1c.append(t1)

            for st in range(N_ST):
                b0 = st * ST
                # -- transpose x to feature-major chunks xt_k [fw, ST] --
                xt_k = []
                for k, (f0, fw) in enumerate(FCH):
                    xt = xt_pool.tile([fw, ST], F32R, name=f"xt_{k}", tag=f"xt{k}")
                    xt_k.append(xt)
                for bs in range(ST // 128):
                    xin_t = xin_pool.tile([128, FEAT], F32, name="xin_t", tag="xin")
                    nc.sync.dma_start(xin_t[:], xin[b0 + bs * 128 : b0 + (bs + 1) * 128, :])
                    for k, (f0, fw) in enumerate(FCH):
                        pt = pt_pool.tile([fw, 128], F32, name="pt", tag="pt")
                        nc.tensor.transpose(pt[:], xin_t[:, f0 : f0 + fw], identity[:])
                        nc.vector.tensor_copy(xt_k[k][:, bs * 128 : (bs + 1) * 128], pt[:])
                # -- mlp1: h1T chunks [100, ST] --
                h1_m = []
                for m in range(3):
                    pm = pm1_pool.tile([100, ST], F32, name="pm", tag="pm1")
                    for k in range(4):
                        nc.tensor.matmul(
                            pm[:], wefft_k[k][:, m * 100 : (m + 1) * 100], xt_k[k][:],
                            start=(k == 0), stop=False,
                        )
                    nc.tensor.matmul(
                        pm[:], befft_t[:, m * 100 : (m + 1) * 100], ones[:],
                        start=False, stop=True,
                    )
                    h1 = h1_pool.tile([100, ST], F32R, name=f"h1_{m}", tag=f"h1{m}")
                    nc.scalar.activation(h1[:], pm[:], TANH)
                    h1_m.append(h1)
                # -- mlp2 -> asb [101, 2*ST]: cols [0:ST]=a, [ST:2ST]=sb, row 100=ones --
                pm2 = pf_pool.tile([100, 2 * ST], F32, name="pm2", tag="fp")
                for half in range(2):
                    cs = slice(half * 100, (half + 1) * 100)
                    w = slice(half * ST, (half + 1) * ST)
                    for k in range(3):
                        nc.tensor.matmul(
                            pm2[:, w], w2efft_k[k][:, cs], h1_m[k][:],
                            start=(k == 0), stop=False,
                        )
                    nc.tensor.matmul(
                        pm2[:, w], b2efft_t[:, cs], ones[:], start=False, stop=True
                    )
                asb = asb_pool.tile([S + 2, 2 * ST], F32R, name="asb", tag="asb")
                nc.sync.dma_start(asb[S : S + 2, :], onesd[:, :])  # bias rows
                nc.scalar.activation(asb[0:S, :], pm2[:], TANH)
                # -- pairwise head --
                for bs in range(ST // 128):
                    ca = bs * 128
                    cb = ST + bs * 128
                    for c, (c0, cw) in enumerate(CHUNKS):
                        pf = pf_pool.tile([128, cw], F32, name="pf", tag="fp")
                        for w0 in range(0, cw, 512):
                            ww = min(512, cw - w0)
                            nc.tensor.matmul(
                                pf[:, w0 : w0 + ww], asb[0 : S + 2, ca : ca + 128],
                                m0c[c][:, w0 : w0 + ww], start=True, stop=False,
                            )
                            nc.tensor.matmul(
                                pf[:, w0 : w0 + ww], asb[0:S, cb : cb + 128],
                                m1c[c][:, w0 : w0 + ww], start=False, stop=True,
                            )
                        ot = out_pool.tile([128, cw], F32, name="ot", tag="ot")
                        nc.scalar.activation(ot[:], pf[:], TANH)
                        nc.sync.dma_start(
                            yout[b0 + bs * 128 : b0 + (bs + 1) * 128, c0 : c0 + cw], ot[:]
                        )

    nc.compile()
    _module_cache = nc
    return nc


def _run(inputs, trace=False, trace_cores=None):
    nc = _build_module()
    hw = _host_weights(inputs)
    x = np.ascontiguousarray(np.asarray(inputs["x"], np.float32))
    in_maps = []
    for c in range(N_CORES):
        m = dict(hw)
        m["xin"] = x[c * BL : (c + 1) * BL]
        in_maps.append(m)
    kwargs = {}
    if trace:
        bass_utils.upload_artifacts = lambda tmpdir: tmpdir  # no cloud store here
        kwargs = dict(trace=True, trace_cores=trace_cores or [0])
    res = bass_utils.run_bass_kernel_spmd(
        nc, in_maps, core_ids=list(range(N_CORES)), **kwargs
    )
    out = np.concatenate([res.results[c]["yout"] for c in range(N_CORES)], axis=0)
    return out, res


def kernel(**inputs) -> np.ndarray:
    out, _ = _run(inputs)
    return out


# revision 8
# speedup vs baseline: 1.1372x; 1.1372x over previous
"""Trainium2 Bass kernel for nn_DQN_57904749085018 (gnn_message_passing).

Computation (reference semantics):
    g   = x[:, idx]                                  [B, S, L] gather
    h   = (g - mean) * rsqrt(var+eps) * gamma + beta [B, S, L] batchnorm (eval)
    h1  = tanh(einsum('bsl,sol->bso', h, W1) + b1)   [B, S, 3]
    h2  = tanh(einsum('bsk,sok->bso', h1, W2) + b2)  [B, S, 2]
    a, sb = h2[..., 0], h2[..., 1]
    out[b,i,j] = tanh(a[b,i]*W3[i,j,0] + sb[b,j]*W3[i,j,1] + b3[i,j])
    -> reshape [B, S*S]

Kernel strategy (pure data parallel over 8 cores, batch-sharded):
  * gather + batchnorm + Linear1 fold into ONE dense matmul x @ Weff.T
    (Weff host-precomputed from idx/gamma/beta/mean/var/W1 - it is tiny).
  * the pairwise head is expressed as two block-structured matmuls:
      z = a' @ M0' + sb @ M1   with  M0'[k, i*S+j] = d_{k,i} W3[i,j,0],
      M0'[S, i*S+j] = b3[i,j] (ones-row trick), M1[k, i*S+j] = d_{k,j} W3[i,j,1]
    then out = tanh(z) on the scalar engine.
  * all matmuls run in float32r (full-rate PE); accumulation is fp32 in PSUM.
  * per 512-batch super-tile: PE-transpose x to feature-major, 2 small MLP
    stages, then 2 matmuls + 1 tanh + 1 DMA per 1024-column output chunk.
"""

import sys

import numpy as np

if "/opt/trn_rl_repo" not in sys.path:
    sys.path.insert(0, "/opt/trn_rl_repo")

import concourse.bacc as bacc
import concourse.mybir as mybir
from concourse import bass_utils
from concourse.masks import make_identity
from concourse.tile import TileContext

S = 100
L = 13
FEAT = 4 * S + 7  # 407
B = 8192
EPS = 1e-5
N_CORES = 8
BL = B // N_CORES  # 1024 batch rows per core
ST = 512  # batch super-tile (matmul moving dim)
N_ST = BL // ST  # 2
SS = S * S  # 10000
CHUNK = 1024  # output column chunk (2 PSUM banks)
CHUNKS = [(c * CHUNK, min(CHUNK, SS - c * CHUNK)) for c in range((SS + CHUNK - 1) // CHUNK)]
FCH = [(0, 128), (128, 128), (256, 128), (384, FEAT - 384)]  # feature chunks
F32R = mybir.dt.float32r
F32 = mybir.dt.float32

_module_cache = None


def _build_indices():
    idx = [[2 * i, 2 * i + 1] for i in range(S)]
    start = 2 * S
    for k in range(S):
        u, v = k, (k + 1) % S
        idx[u].extend([start, start + 1])
        idx[v].extend([start, start + 1])
        start += 2
    g0 = 4 * S
    for i in range(S):
        idx[i].extend(range(g0, g0 + 7))
    return np.asarray(idx, dtype=np.int64)


def _host_weights(inputs):
    f64 = np.float64
    gamma = np.asarray(inputs["gamma"], f64)
    beta = np.asarray(inputs["beta"], f64)
    mean = np.asarray(inputs["mean"], f64)
    var = np.asarray(inputs["var"], f64)
    W1 = np.asarray(inputs["W1"], f64)  # [S, 3, L]
    b1 = np.asarray(inputs["b1"], f64)  # [S, 3]
    W2 = np.asarray(inputs["W2"], f64)  # [S, 2, 3]
    b2 = np.asarray(inputs["b2"], f64)  # [S, 2]
    W3 = np.asarray(inputs["W3"], f64)  # [S, S, 2]
    b3 = np.asarray(inputs["b3"], f64)  # [S, S]
    idx = np.asarray(inputs["idx"], np.int64)  # [S, L]

    scale = gamma / np.sqrt(var + EPS)  # [S, L]
    shift = beta - mean * scale  # [S, L]

    # Weff[(s,o), f] = sum_l [idx[s,l]==f] W1[s,o,l]*scale[s,l]
    Wsc = W1 * scale[:, None, :]  # [S, 3, L]
    Weff = np.zeros((S, 3, FEAT), f64)
    s_ix = np.repeat(np.arange(S), 3 * L)
    o_ix = np.tile(np.repeat(np.arange(3), L), S)
    f_ix = np.repeat(idx[:, None, :], 3, axis=1).ravel()
    np.add.at(Weff, (s_ix, o_ix, f_ix), Wsc.ravel())
    Weff = Weff.reshape(3 * S, FEAT)
    beff = (b1 + np.einsum("sol,sl->so", W1, shift)).reshape(3 * S)

    # W2eff[(o2*S+s), (s*3+k)] = W2[s, o2, k]
    W2eff = np.zeros((2 * S, 3 * S), f64)
    for s in range(S):
        for o2 in range(2):
            W2eff[o2 * S + s, s * 3 : s * 3 + 3] = W2[s, o2, :]
    b2eff = b2.T.reshape(2 * S)  # [o2*S+s]

    # M0p[k, i*S+j] = d_{k,i} W3[i,j,0]; row S = b3 flat. M1[k, i*S+j] = d_{k,j} W3[i,j,1]
    M0p = np.zeros((S + 2, SS), f64)
    M1 = np.zeros((S, SS), f64)
    cols = np.arange(SS)
    M0p[np.repeat(np.arange(S), S), cols] = W3[:, :, 0].ravel()

    M1[np.tile(np.arange(S), S), cols] = W3[:, :, 1].ravel()

    def split_hl(v):
        # hi keeps 10 mantissa bits (exactly representable at fp32r ingestion);
        # lo carries the residual so the bias survives reduced-precision matmul.
        hi = np.asarray(v, np.float32).view(np.uint32) & np.uint32(0xFFFFE000)
        hi = hi.view(np.float32).astype(f64)
        return hi, np.asarray(v - hi)

    b3_hi, b3_lo = split_hl(b3.ravel())
    beff_hi, beff_lo = split_hl(beff)
    b2eff_hi, b2eff_lo = split_hl(b2eff)

    M0p[S, :] = b3_hi
    M0p[S + 1, :] = b3_lo

    # pack per output chunk: [M0p chunk | M1 chunk] side by side
    mw = np.zeros((S + 2, 2 * SS), f64)
    for c0, cw in CHUNKS:
        mw[:, 2 * c0 : 2 * c0 + cw] = M0p[:, c0 : c0 + cw]
        mw[0:S, 2 * c0 + cw : 2 * c0 + 2 * cw] = M1[:, c0 : c0 + cw]

    c32 = lambda a: np.ascontiguousarray(a, dtype=np.float32)
    return {
        "wefft": c32(Weff.T),  # [FEAT, 300]
        "befft": c32(np.stack([beff_hi, beff_lo])),  # [2, 300]
        "w2efft": c32(W2eff.T),  # [300, 200]
        "b2efft": c32(np.stack([b2eff_hi, b2eff_lo])),  # [2, 200]
        "mw": c32(mw),  # [102, 2*SS] packed pairwise weights
        "onesd": np.ones((2, 2 * ST), np.float32),
    }


def _build_module():
    global _module_cache
    if _module_cache is not None:
        return _module_cache

    nc = bacc.Bacc("TRN2", target_bir_lowering=False, debug=False, num_devices=N_CORES)
    xin = nc.dram_tensor("xin", [BL, FEAT], F32, kind="ExternalInput").ap()
    wefft = nc.dram_tensor("wefft", [FEAT, 300], F32R, kind="ExternalInput").ap()
    befft = nc.dram_tensor("befft", [2, 300], F32R, kind="ExternalInput").ap()
    w2efft = nc.dram_tensor("w2efft", [300, 200], F32R, kind="ExternalInput").ap()
    b2efft = nc.dram_tensor("b2efft", [2, 200], F32R, kind="ExternalInput").ap()
    mw = nc.dram_tensor("mw", [S + 2, 2 * SS], F32R, kind="ExternalInput").ap()
    onesd = nc.dram_tensor("onesd", [2, 2 * ST], F32R, kind="ExternalInput").ap()
    yout = nc.dram_tensor("yout", [BL, SS], F32, kind="ExternalOutput").ap()

    TANH = mybir.ActivationFunctionType.Tanh
    PAIRS = [(CHUNKS[2 * p], CHUNKS[2 * p + 1]) for p in range(len(CHUNKS) // 2)]

    with TileContext(nc) as tc:
        with (
            tc.tile_pool(name="const", bufs=1) as const,
            tc.tile_pool(name="xin_pool", bufs=2) as xin_pool,
            tc.tile_pool(name="xt_pool", bufs=2) as xt_pool,
            tc.tile_pool(name="h1_pool", bufs=2) as h1_pool,
            tc.tile_pool(name="asb_pool", bufs=2) as asb_pool,
            tc.tile_pool(name="out_pool", bufs=4) as out_pool,
            tc.tile_pool(name="ps_pool", bufs=2, space="PSUM") as ps_pool,
            tc.tile_pool(name="pf_pool", bufs=3, space="PSUM") as pf_pool,
        ):
            # --- small constants (fast, issued first) ---
            identity = const.tile([128, 128], F32)
            make_identity(nc, identity)
            ones2 = const.tile([2, 2 * ST], F32R)
            nc.gpsimd.dma_start(ones2[:], onesd[:, :])
            ones = ones2[:, 0:ST]  # [2, ST] - K=2 rhs for hi/lo bias matmuls
            warm = const.tile([1, 8], F32)
            nc.scalar.activation(warm[:], ones2[0:1, 0:8], TANH)  # preload tanh table

            wefft_k = []
            for k, (f0, fw) in enumerate(FCH):
                t = const.tile([fw, 3 * S], F32R, name=f"wefft_{k}")
                nc.gpsimd.dma_start(t[:], wefft[f0 : f0 + fw, :])
                wefft_k.append(t)
            befft_t = const.tile([2, 3 * S], F32R)
            nc.gpsimd.dma_start(befft_t[:], befft[:, :])
            w2efft_k = []
            for k in range(3):
                t = const.tile([100, 2 * S], F32R, name=f"w2efft_{k}")
                nc.gpsimd.dma_start(t[:], w2efft[k * 100 : (k + 1) * 100, :])
                w2efft_k.append(t)
            b2efft_t = const.tile([2, 2 * S], F32R)
            nc.gpsimd.dma_start(b2efft_t[:], b2efft[:, :])

            def emit_front(st):
                b0 = st * ST
                xt_k = []
                for k, (f0, fw) in enumerate(FCH):
                    xt = xt_pool.tile([fw, ST], F32R, name=f"xt_{k}", tag=f"xt{k}")
                    xt_k.append(xt)
                for bs in range(ST // 128):
                    xin_t = xin_pool.tile([128, FEAT], F32, name="xin_t", tag="xin")
                    nc.gpsimd.dma_start(xin_t[:], xin[b0 + bs * 128 : b0 + (bs + 1) * 128, :])
                    for k, (f0, fw) in enumerate(FCH):
                        pt = ps_pool.tile([fw, 128], F32, name="pt", tag="ps")
                        nc.tensor.transpose(pt[:], xin_t[:, f0 : f0 + fw], identity[:])
                        nc.vector.tensor_copy(xt_k[k][:, bs * 128 : (bs + 1) * 128], pt[:])
                h1_m = []
                for m in range(3):
                    pm = ps_pool.tile([100, ST], F32, name="pm", tag="ps")
                    for k in range(4):
                        nc.tensor.matmul(
                            pm[:], wefft_k[k][:, m * 100 : (m + 1) * 100], xt_k[k][:],
                            start=(k == 0), stop=False,
                        )
                    nc.tensor.matmul(
                        pm[:], befft_t[:, m * 100 : (m + 1) * 100], ones[:],
                        start=False, stop=True,
                    )
                    h1 = h1_pool.tile([100, ST], F32R, name=f"h1_{m}", tag=f"h1{m}")
                    nc.scalar.activation(h1[:], pm[:], TANH)
                    h1_m.append(h1)
                pm2 = pf_pool.tile([100, 2 * ST], F32, name="pm2", tag="fp")
                for half in range(2):
                    cs = slice(half * 100, (half + 1) * 100)
                    w = slice(half * ST, (half + 1) * ST)
                    for k in range(3):
                        nc.tensor.matmul(
                            pm2[:, w], w2efft_k[k][:, cs], h1_m[k][:],
                            start=(k == 0), stop=False,
                        )
                    nc.tensor.matmul(
                        pm2[:, w], b2efft_t[:, cs], ones[:], start=False, stop=True
                    )
                asb = asb_pool.tile([S + 2, 2 * ST], F32R, name="asb", tag="asb")
                nc.gpsimd.dma_start(asb[S : S + 2, :], onesd[:, :])  # ones bias rows
                nc.scalar.activation(asb[0:S, :], pm2[:], TANH)
                return asb

            def emit_final(st, asb):
                b0 = st * ST
                for bs in range(ST // 128):
                    ca = bs * 128
                    cb = ST + bs * 128
                    for (c0A, cwA), (c0B, cwB) in PAIRS:
                        pfs = []
                        for c0, cw in ((c0A, cwA), (c0B, cwB)):
                            pf = pf_pool.tile([128, cw], F32, name="pf", tag="fp")
                            mwc = mw_c[c0]
                            for w0 in range(0, cw, 512):
                                ww = min(512, cw - w0)
                                nc.tensor.matmul(
                                    pf[:, w0 : w0 + ww], asb[0 : S + 2, ca : ca + 128],
                                    mwc[0 : S + 2, w0 : w0 + ww], start=True, stop=False,
                                )
                                nc.tensor.matmul(
                                    pf[:, w0 : w0 + ww], asb[0:S, cb : cb + 128],
                                    mwc[0:S, cw + w0 : cw + w0 + ww], start=False, stop=True,
                                )
                            pfs.append(pf)
                        ot = out_pool.tile([128, cwA + cwB], F32, name="ot", tag="ot")
                        nc.scalar.activation(ot[:, 0:cwA], pfs[0][:], TANH)
                        nc.scalar.activation(ot[:, cwA : cwA + cwB], pfs[1][:], TANH)
                        nc.sync.dma_start(
                            yout[b0 + bs * 128 : b0 + (bs + 1) * 128, c0A : c0A + cwA + cwB],
                            ot[:],
                        )

            # super-tile 0 front first (overlaps the big pairwise-weight DMA)
            asb0 = emit_front(0)
            mw_c = {}
            for c0, cw in CHUNKS:
                t = const.tile([S + 2, 2 * cw], F32R, name=f"mw_{c0}")
                nc.gpsimd.dma_start(t[:], mw[:, 2 * c0 : 2 * c0 + 2 * cw])
                mw_c[c0] = t
            emit_final(0, asb0)
            asb1 = emit_front(1)
            emit_final(1, asb1)

    nc.compile()
    _module_cache = nc
    return nc


def _run(inputs, trace=False, trace_cores=None):
    nc = _build_module()
    hw = _host_weights(inputs)
    x = np.ascontiguousarray(np.asarray(inputs["x"], np.float32))
    in_maps = []
    for c in range(N_CORES):
        m = dict(hw)
        m["xin"] = x[c * BL : (c + 1) * BL]
        in_maps.append(m)
    kwargs = {}
    if trace:
        bass_utils.upload_artifacts = lambda tmpdir: tmpdir  # no cloud store here
        kwargs = dict(trace=True, trace_cores=trace_cores or [0])
    res = bass_utils.run_bass_kernel_spmd(
        nc, in_maps, core_ids=list(range(N_CORES)), **kwargs
    )
    out = np.concatenate([res.results[c]["yout"] for c in range(N_CORES)], axis=0)
    return out, res


def kernel(**inputs) -> np.ndarray:
    out, _ = _run(inputs)
    return out
